# revision 1
# baseline (speedup 1.0000x reference)
"""Trainium2 Bass kernel for nn_DeepTropNet (dense tropical transformer).

Strategy:
- Batch-parallel across cores (B=4 -> cores 0..3; cores 4..7 duplicate).
- Residual stream kept TRANSPOSED in SBUF: hT [D=128 partitions, L=197 free],
  so every projection is a natural TensorE matmul (contraction on partitions).
- All tropical (max-plus) contractions use the log-sum-exp trick at low
  temperature: max_i(a_i+b_i) = T*log(sum_i e^{a_i/T} e^{b_i/T}), separable ->
  a TensorE matmul of elementwise exponentials. Stabilizers are data-derived
  maxima (host-side for weights, on-device for activations).
- All weight transposes/exp-tables/bias-folds are host-side numpy prep.
- Custom DVE ops (affine_then_max/min) fuse the piecewise-linear fold.
"""
import sys

sys.path.insert(0, "/opt/trn_rl_repo")

import numpy as np

import concourse.bass as bass
import concourse.tile as tile
from concourse import bacc, mybir
from concourse import bass_utils

FP = mybir.dt.float32
BF = mybir.dt.bfloat16
AX = mybir.AxisListType
OP = mybir.AluOpType
ACTF = mybir.ActivationFunctionType

NL, D, H, DK, L, F, P, NC, PS = 2, 128, 8, 16, 197, 256, 8, 1000, 16
EPS = 1e-5
SCALE = DK ** -0.5
T1 = 0.01        # temperature for D=128-contraction tropical matmuls (Q,K,z)
T2 = 0.06        # temperature for DK=16-contraction tropical scores
NPATCH = 196
KCH = 6          # 768/128 contraction chunks for patch embed
KT0, KT1 = 128, L - 128   # score k-tiles
LC = 99          # own query/token columns per core (2-way split, overlap @98)
RG = [[0, 1], [2, 3], [4, 5], [6, 7]]   # core pairs (one image each)
import os
_FAKE_GATHER = bool(int(os.environ.get("ANT_FAKE_GATHER", "0")))


# ---------------------------------------------------------------- custom DVE ops
def _make_op(name, body_fn, reference):
    from concourse.dve_spec import Spec, lower, _has_src1
    from concourse.dve_uop import DveOpSpec
    import concourse.dve_ops as dve_ops

    for o in dve_ops.OPS:
        if o.name == name:
            return o
    spec = Spec(body=body_fn(), reference=reference)
    row = dve_ops._CUSTOM_DVE_ROW_BASE + len(dve_ops.OPS)
    assert row < 0x20
    dve_ops._SUB_OPCODE_FOR_NAME[name] = row
    shas = {}
    for ver in ("v3", "v4"):
        try:
            uops = lower(spec, ver=ver)
            shas[ver] = DveOpSpec(name=name, opcode=row, uops=uops,
                                  rd1_en=_has_src1(spec)).sha(ver)
        except Exception:
            pass
    op = dve_ops.DveOp(name, spec, subdim=False, uops_sha=shas)
    dve_ops.OPS.append(op)
    dve_ops.CUSTOM_DVE_SPECS[name] = spec
    return op


def _register_ops():
    from concourse.dve_spec import C0, C1, Src0, Src1, maxx, minn

    aff_max = _make_op(
        "ANT_AFFINE_THEN_MAX",
        lambda: maxx(Src0 * C0 + C1, Src1),
        lambda in0, in1, s0, s1, imm2: np.maximum(
            (in0.astype(np.float32) * s0 + s1), in1).astype(np.float32),
    )
    aff_min = _make_op(
        "ANT_AFFINE_THEN_MIN",
        lambda: minn(Src0 * C0 + C1, Src1),
        lambda in0, in1, s0, s1, imm2: np.minimum(
            (in0.astype(np.float32) * s0 + s1), in1).astype(np.float32),
    )
    sub_aff = _make_op(
        "ANT_SUB_THEN_AFFINE",
        lambda: (Src0 - Src1) * C0 + C1,
        lambda in0, in1, s0, s1, imm2: (
            (in0.astype(np.float32) - in1) * s0 + s1).astype(np.float32),
    )
    return aff_max, aff_min, sub_aff


AFF_MAX, AFF_MIN, SUB_AFF = _register_ops()


# Bind Exp and Ln to the one ACT table set that holds both, so the score
# loop's Ln/Exp alternation doesn't reload tables (~2.7us per switch).
def _patch_act_tables():
    import concourse.hw_specs as hw_specs
    import concourse.bacc as bacc_mod
    if getattr(hw_specs, "_ant_combo_patched", False):
        return
    orig = hw_specs.get_activation_tables

    def patched(arch):
        tabs = orig(arch)
        A = mybir.ActivationFunctionType
        combo = "natural_log_exp_and_others"
        if combo in tabs and A.Exp in tabs[combo] and A.Ln in tabs[combo]:
            for name, fns in tabs.items():
                if name != combo:
                    fns.discard(A.Exp)
                    fns.discard(A.Ln)
        return tabs

    hw_specs.get_activation_tables = patched
    bacc_mod.get_activation_tables = patched
    hw_specs._ant_combo_patched = True


_patch_act_tables()


# ---------------------------------------------------------------- host-side prep
class _Pack:
    def __init__(self):
        self.cols = {}
        self.n = 0

    def add(self, name, ncols):
        self.cols[name] = (self.n, ncols)
        self.n += ncols

    def sl(self, name):
        return self.cols[name]


def _layout():
    pk = _Pack()
    pk.add("b0", LC)
    pk.add("onescol", 1)
    pk.add("meancol", 1)              # 1/128
    pk.add("onesrow", 128)            # all ones; row 0 used as [1,128] lhsT
    pk.add("epscol", 1)
    pk.add("clampcol", 1)
    pk.add("zerocol", 1)
    pk.add("padneg", 1)
    pk.add("t1col", 1)
    for i in range(NL):
        s = f"_{i}"
        pk.add("ln1wb" + s, 2)
        pk.add("ln2wb" + s, 2)
        pk.add("cq" + s, 2)
        pk.add("ck" + s, 2)
        pk.add("gb8n" + s, 1)         # rows 0:8: -gate_b
        pk.add("st8" + s, 1)          # rows 0:8: scale*T2/temp_h
        pk.add("svec" + s, 2)         # per-group scale/temp cols (pad rows 0)
        pk.add("svecn" + s, 2)        # -svec
        pk.add("bo2" + s, 1)
        pk.add("ctu" + s, 2)
        pk.add("lfa" + s, 16)         # col t*8+p
        pk.add("lfc" + s, 16)
        pk.add("glc" + s, 2)
        pk.add("glc1m" + s, 2)
        pk.add("cub" + s, 2)
        pk.add("cubn" + s, 2)         # -1.702*cu_b (gelu sigmoid-approx)
        pk.add("fgbn" + s, 2)         # -fg_b
        pk.add("dnb" + s, 1)
    pk.add("fnwb", 2)
    pk.add("hwT", 1024)
    pk.add("hb", 8)
    return pk


def _layout_b():
    pk = _Pack()
    pk.add("xp", KCH * LC)            # per-(batch,rank) patch window chunks
    pk.add("pw", KCH * 128)
    pk.add("meancolb", 1)
    pk.add("e8", 8 * 8)               # unit-column blocks for sigma matmuls
    pk.add("segg", 2 * 128)           # per-group head segment lhsT (rows 0:8)
    pk.add("oneh", 8 * 128)           # row-h all-ones lhsT blocks (rows 0:8)
    pk.add("onesrowb", 128)
    for i in range(NL):
        s = f"_{i}"
        pk.add("wqexpT" + s, 256)
        pk.add("wkexpT" + s, 256)
        pk.add("wvT" + s, 256)
        pk.add("gwT" + s, 8)
        pk.add("woT" + s, 256)
        pk.add("tuexpT" + s, F)
        pk.add("cuT" + s, F)
        pk.add("fgT" + s, F)
        pk.add("dnT" + s, 2 * 128)
    return pk


PK = _layout()
NCOL = PK.n
PKB = _layout_b()
NCOLB = PKB.n


def _prep_pack(inp):
    W = np.zeros((128, NCOL), np.float32)
    WB = np.zeros((128, NCOLB), np.float32)

    def put(name, arr):
        o, n = PK.sl(name)
        a = np.asarray(arr, np.float32)
        assert a.ndim == 2 and a.shape[1] == n, (name, a.shape, n)
        W[: a.shape[0], o:o + n] = a

    def putb(name, arr):
        o, n = PKB.sl(name)
        a = np.asarray(arr, np.float32)
        assert a.ndim == 2 and a.shape[1] == n, (name, a.shape, n)
        WB[: a.shape[0], o:o + n] = a

    bn_s = inp["bn_gamma"] / (inp["bn_run_range"] + EPS)
    bn_b = inp["bn_beta"] - inp["bn_run_max"] * bn_s
    pos = inp["pos_embed"][0]                                     # [L, D]
    clsb = inp["cls_token"].reshape(D)
    b0 = np.empty((D, L), np.float32)
    b0[:, 0] = bn_s * (clsb + pos[0]) + bn_b
    b0[:, 1:] = (bn_s[:, None] * (inp["patch_b"][:, None] + pos[1:].T)
                 + bn_b[:, None])
    # b0 own-slice is filled per core in build_in_maps

    pwT = (bn_s[:, None] * inp["patch_w"]).T                      # [768, D]
    putb("pw", np.concatenate([pwT[128 * k:128 * (k + 1)] for k in range(KCH)], 1))

    put("onescol", np.ones((128, 1)))
    put("meancol", np.full((128, 1), 1.0 / D))
    putb("meancolb", np.full((128, 1), 1.0 / D))
    e8 = np.zeros((128, 64), np.float32)
    for h in range(H):
        e8[:, 8 * h + h] = 1.0
    putb("e8", e8)
    segg = np.zeros((8, 2 * 128), np.float32)
    for g in range(2):
        for j in range(4):
            segg[4 * g + j, 128 * g + 32 * j:128 * g + 32 * j + DK] = 1.0
    putb("segg", segg)
    oneh = np.zeros((8, 8 * 128), np.float32)
    for h in range(H):
        oneh[h, 128 * h:128 * (h + 1)] = 1.0
    putb("oneh", oneh)
    put("onesrow", np.ones((128, 128)))
    putb("onesrowb", np.ones((128, 128)))
    put("epscol", np.full((128, 1), EPS))
    put("clampcol", np.full((128, 1), 1e-30))
    put("zerocol", np.zeros((128, 1)))
    padneg = np.zeros((128, 1), np.float32)
    for j in range(4):
        padneg[32 * j + DK:32 * (j + 1)] = -1e5
    put("padneg", padneg)
    put("t1col", np.full((128, 1), T1))

    def pad32_rows(vec):
        # [D] head-indexed (16h+d) -> two [128] group columns at rows 32j+d
        out = np.zeros((2, 128), np.float32)
        for h in range(H):
            g, j = divmod(h, 4)
            out[g, 32 * j:32 * j + DK] = vec[DK * h:DK * (h + 1)]
        return out

    for i in range(NL):
        s = f"_{i}"
        put("ln1wb" + s, np.stack([inp["n1_w"][i], inp["n1_b"][i]], 1))
        put("ln2wb" + s, np.stack([inp["n2_w"][i], inp["n2_b"][i]], 1))
        for nm, bkey in (("wq", "bq"), ("wk", "bk")):
            Wt = inp[nm][i]
            mW = Wt.max(1)
            we = np.exp((Wt - mW[:, None]) / T1).T          # [i, o=16h+d]
            wep = np.zeros((128, 256), np.float32)
            for h in range(H):
                g, j = divmod(h, 4)
                wep[:, 128 * g + 32 * j:128 * g + 32 * j + DK] = \
                    we[:, DK * h:DK * (h + 1)]
            putb(nm + "expT" + s, wep)
            put("c" + nm[1] + s, pad32_rows(mW + inp[bkey][i]).T)
        wvp = np.zeros((128, 256), np.float32)
        wvT_ = inp["wv"][i].T                               # [i, 16h+d]
        for h in range(H):
            g, j = divmod(h, 4)
            wvp[:, 128 * g + 32 * j:128 * g + 32 * j + DK] = \
                wvT_[:, DK * h:DK * (h + 1)]
        putb("wvT" + s, wvp)
        putb("gwT" + s, inp["gate_w"][i].T)
        put("gb8n" + s, -inp["gate_b"][i][:, None])
        put("st8" + s, (SCALE * T2 / inp["temp"][i])[:, None])
        svec_ = pad32_rows(np.repeat(SCALE / inp["temp"][i], DK)).T
        put("svec" + s, svec_)
        put("svecn" + s, -svec_)
        woT_ = inp["wo"][i].T                               # [dD=16h+d, o2]
        wop = np.zeros((128, 256), np.float32)
        for h in range(H):
            g, j = divmod(h, 4)
            wop[32 * j:32 * j + DK, 128 * g:128 * (g + 1)] = \
                woT_[DK * h:DK * (h + 1), :]
        putb("woT" + s, wop)
        put("bo2" + s, (inp["bo"][i] + inp["wo"][i] @ inp["bv"][i])[:, None])
        tu = inp["tu_w"][i]
        mtu = tu.max(1)
        putb("tuexpT" + s, np.exp((tu - mtu[:, None]) / T1).T)
        ctu = mtu + inp["tu_b"][i]
        put("ctu" + s, np.stack([ctu[:128], ctu[128:]], 1))
        lfa = np.zeros((128, 16), np.float32)
        lfc = np.zeros((128, 16), np.float32)
        for t in range(2):
            for p in range(P):
                lfa[:, t * 8 + p] = inp["lf_a"][i][p, 128 * t:128 * (t + 1)]
                lfc[:, t * 8 + p] = inp["lf_c"][i][p, 128 * t:128 * (t + 1)]
        put("lfa" + s, lfa)
        put("lfc" + s, lfc)
        gl = 1.0 / (1.0 + np.exp(-inp["lf_gate"][i]))
        put("glc" + s, np.stack([gl[:128], gl[128:]], 1))
        put("glc1m" + s, np.stack([1 - gl[:128], 1 - gl[128:]], 1))
        putb("cuT" + s, inp["cu_w"][i].T)
        put("cub" + s, np.stack([inp["cu_b"][i][:128], inp["cu_b"][i][128:]], 1))
        cbn = -1.702 * inp["cu_b"][i]
        put("cubn" + s, np.stack([cbn[:128], cbn[128:]], 1))
        putb("fgT" + s, inp["fg_w"][i].T)
        put("fgbn" + s, np.stack([-inp["fg_b"][i][:128], -inp["fg_b"][i][128:]], 1))
        dnT = inp["dn_w"][i].T                                    # [F, D]
        putb("dnT" + s, np.concatenate([dnT[:128], dnT[128:]], 1))
        put("dnb" + s, inp["dn_b"][i][:, None])

    put("fnwb", np.stack([inp["fn_w"], inp["fn_b"]], 1))
    hwT = np.zeros((128, 1024), np.float32)
    hb = np.zeros((128, 8), np.float32)
    hw_pad = np.zeros((1024, D), np.float32)
    hw_pad[:NC] = inp["head_w"]
    hb_pad = np.zeros(1024, np.float32)
    hb_pad[:NC] = inp["head_b"]
    for j in range(8):
        hwT[:, 128 * j:128 * (j + 1)] = hw_pad[128 * j:128 * (j + 1)].T
        hb[:, j] = hb_pad[128 * j:128 * (j + 1)]
    put("hwT", hwT)
    put("hb", hb)
    return W, WB, b0


def _prep_x(inp, b, rank):
    xb = inp["x"][b]
    xp = xb.reshape(3, 14, PS, 14, PS).transpose(1, 3, 0, 2, 4).reshape(
        NPATCH, 3 * PS * PS)
    # window of LC patch columns matching own tokens; even rank owns tokens
    # 0..98 (CLS + patches 0..97 -> leading zero col), odd owns 98..196
    # (patches 97..195)
    win = np.zeros((LC, 3 * PS * PS), np.float32)
    if rank == 0:
        win[1:] = xp[0:98]
    else:
        win[:] = xp[97:196]
    xpT = np.ascontiguousarray(win.T.astype(np.float32))
    return np.concatenate([xpT[128 * k:128 * (k + 1)] for k in range(KCH)], 1)


# ---------------------------------------------------------------- bass program
def _build_program(reps=1):
    nc = bacc.Bacc("TRN2", target_bir_lowering=False, debug=False,
                   enable_asserts=True, num_devices=8)
    inp_d = nc.dram_tensor("inp", [128, NCOL], FP, kind="ExternalInput").ap()
    inpb_d = nc.dram_tensor("inpb", [128, NCOLB], BF, kind="ExternalInput").ap()
    out_d = nc.dram_tensor("out", [128, 8], FP, kind="ExternalOutput").ap()
    with tile.TileContext(nc) as tc:
        _bass_body(nc, tc, inp_d, inpb_d, out_d, reps=reps)
    nc.compile()
    return nc


def _bass_body(nc, tc, inp_d, inpb_d, out_d, reps=1):
    import contextlib
    ctx = contextlib.ExitStack()
    perm = ctx.enter_context(tc.tile_pool(name="perm", bufs=1))
    work = ctx.enter_context(tc.tile_pool(name="work", bufs=2))
    psp = ctx.enter_context(tc.tile_pool(name="psp", bufs=1, space="PSUM"))
    dram = ctx.enter_context(tc.tile_pool(name="dram", bufs=1, space="DRAM"))

    WPK = perm.tile([128, NCOL], FP)
    nc.sync.dma_start(WPK[:], inp_d[:])
    WB = perm.tile([128, NCOLB], BF)
    nc.sync.dma_start(WB[:], inpb_d[:])

    def wp(name, rows=128):
        o, n = PK.sl(name)
        return WPK[0:rows, o:o + n]

    def col(name, j=0, rows=128):
        o, n = PK.sl(name)
        return WPK[0:rows, o + j:o + j + 1]

    def wpb(name, rows=128):
        o, n = PKB.sl(name)
        return WB[0:rows, o:o + n]

    _pp_ctr = [0]

    def pp(shape, tag="pp", bufs=3):
        _pp_ctr[0] += 1
        return psp.tile(shape, FP, tag=tag, bufs=bufs,
                        name=f"{tag}{_pp_ctr[0]}",
                        padded_shape=[128, 512])

    onesrow = wp("onesrow")[0:1, :]
    onesrowb = wpb("onesrowb")[0:1, :]
    meancol = wp("meancol")

    hT = perm.tile([128, LC], FP)    # residual (transposed, own tokens), fp32

    if reps > 1:
        loop_cm = tc.For_i(0, reps, 1)
        loop_cm.__enter__()

    # ---- patch embed (bf16, own token window) ----
    pe = pp([128, LC])
    xo, _ = PKB.sl("xp")
    po, _ = PKB.sl("pw")
    for k in range(KCH):
        nc.tensor.matmul(pe[:], WB[:, po + 128 * k: po + 128 * (k + 1)],
                         WB[:, xo + LC * k: xo + LC * (k + 1)],
                         start=(k == 0), stop=(k == KCH - 1))
    nc.vector.tensor_tensor(hT[:, 0:LC], pe[:], wp("b0"), op=OP.add)

    def layer_norm(wb_ap, dt):
        hb2 = work.tile([128, 2 * LC], BF, tag="hb2")   # [h, h^2] bf16
        nc.scalar.activation(hb2[:, 0:LC], hT[:, 0:LC], ACTF.Copy)
        nc.vector.tensor_tensor(hb2[:, LC:2 * LC], hb2[:, 0:LC], hb2[:, 0:LC],
                                op=OP.mult)
        stats = pp([1, 2 * LC])
        nc.tensor.matmul(stats[:], wpb("meancolb"), hb2[:], start=True, stop=True)
        mean = work.tile([1, LC], BF, tag="mean")
        nc.vector.tensor_copy(mean[:], stats[0:1, 0:LC])
        msq = work.tile([1, LC], FP, tag="msq")
        nc.vector.tensor_tensor(msq[:], mean[:], mean[:], op=OP.mult)
        var = work.tile([1, LC], FP, tag="var")
        nc.vector.tensor_tensor(var[:], stats[0:1, LC:2 * LC], msq[:],
                                op=OP.subtract)
        lnv = work.tile([1, LC], FP, tag="lnv")
        nc.scalar.activation(lnv[:], var[:], ACTF.Ln, bias=col("epscol", rows=1))
        rstd = work.tile([1, LC], BF, tag="rstd")
        nc.scalar.activation(rstd[:], lnv[:], ACTF.Exp, scale=-0.5)
        mr = work.tile([1, LC], BF, tag="mr")
        nc.vector.tensor_tensor(mr[:], mean[:], rstd[:], op=OP.mult)
        rstdB = pp([128, LC])
        nc.tensor.matmul(rstdB[:], onesrowb, rstd[:], start=True, stop=True)
        mrB = pp([128, LC])
        nc.tensor.matmul(mrB[:], onesrowb, mr[:], start=True, stop=True)
        t1_ = work.tile([128, LC], FP, tag="lnt1")
        nc.vector.tensor_tensor(t1_[:], hT[:, 0:LC], rstdB[:], op=OP.mult)
        t2_ = work.tile([128, LC], FP, tag="lnt2")
        nc.vector.tensor_tensor(t2_[:], t1_[:], mrB[:], op=OP.subtract)
        hn = work.tile([128, LC], dt, tag="hn")
        nc.vector.tensor_scalar(hn[:], t2_[:], wb_ap[:, 0:1], wb_ap[:, 1:2],
                                op0=OP.mult, op1=OP.add)
        return hn

    def trop_exp_rhs(hn, mxB, W):
        from concourse import bass_isa
        nc.gpsimd.partition_all_reduce(mxB[:], hn[:], channels=128,
                                       reduce_op=bass_isa.ReduceOp.max)
        xc = work.tile([128, W], BF, tag=f"xc{W}")
        nc.vector.tensor_tensor(xc[:], hn[:], mxB[:], op=OP.subtract)
        xe = work.tile([128, W], BF, tag=f"xe{W}")
        nc.scalar.activation(xe[:], xc[:], ACTF.Exp, scale=1.0 / T1, bias=col("zerocol"))
        return xe

    def trop_project(xe, mxB, lhsT, cvec, out_tile, W):
        sp = pp([128, W])
        nc.tensor.matmul(sp[:], lhsT, xe[:], start=True, stop=True)
        lg = work.tile([128, W], BF, tag=f"trop_lg{W}")
        nc.scalar.activation(lg[:], sp[:], ACTF.Ln, bias=col("clampcol"))
        nc.vector.tensor_scalar(out_tile[:], lg[:], T1, cvec,
                                op0=OP.mult, op1=OP.add)
        nc.vector.tensor_tensor(out_tile[:], out_tile[:],
                                mxB[:], op=OP.add)

    for i in range(NL):
        s = f"_{i}"
        hn = layer_norm(wp("ln1wb" + s), BF)

        # kick off pair AllGather of hn (other core's token half) early;
        # Q-side work below overlaps the collective
        if _FAKE_GATHER != 2:
            gin = dram.tile([128, LC], BF, name=f"gin{i}")
            gout = dram.tile([256, LC], BF, name=f"gout{i}")
            nc.gpsimd.dma_start(gin[:], hn[:])
            if _FAKE_GATHER == 1:
                nc.gpsimd.dma_start(gout[0:128, :], gin[:])
                nc.gpsimd.dma_start(gout[128:256, :], gin[:])
            else:
                nc.gpsimd.collective_compute(
                    "AllGather", OP.bypass, replica_groups=RG,
                    ins=[gin.opt()], outs=[gout.opt()])

        mxB = work.tile([128, LC], BF, tag="mxBq", name=f"mxA{i}")
        xe = trop_exp_rhs(hn, mxB, LC)

        # Q tropical projections into 32-padded head layout: [128, 2*LC],
        # block g holds heads 4g..4g+3 at partition groups 32j (+16 zero rows)
        Qt = work.tile([128, 2 * LC], BF, tag="Qt")
        for g in range(2):
            trop_project(xe, mxB, wpb("wqexpT" + s)[:, 128 * g:128 * (g + 1)],
                         col("cq" + s, j=g), Qt[:, LC * g:LC * (g + 1)], LC)

        gp = pp([8, LC])
        nc.tensor.matmul(gp[:], wpb("gwT" + s), hn[:], start=True, stop=True)
        ge = work.tile([8, LC], FP, tag="ge")
        nc.scalar.activation(ge[:], gp[:], ACTF.Exp, scale=-1.0,
                             bias=col("gb8n" + s, rows=8))
        gs1 = work.tile([8, LC], FP, tag="gs1")
        nc.vector.tensor_scalar(gs1[:], ge[:], 1.0, None, op0=OP.add)
        gsig = work.tile([8, LC], FP, tag="gsig")
        nc.vector.reciprocal_approx_fast(out=gsig[:], in_=gs1[:])
        gsigb = work.tile([8, LC], BF, tag="gsigb")
        nc.scalar.activation(gsigb[:], gsig[:], ACTF.Copy)
        gts = work.tile([8, LC], BF, tag="gts")
        nc.vector.tensor_scalar(gts[:], gsig[:], col("st8" + s, rows=8), None,
                                op0=OP.mult)

        # Qcs = Qt * (1 - g_seg) * svec  (svec zero on pad rows)
        gsegB = pp([128, 2 * LC])
        for g in range(2):
            nc.tensor.matmul(gsegB[:, LC * g:LC * (g + 1)],
                             wpb("segg", rows=8)[:, 128 * g:128 * (g + 1)],
                             gsigb[:], start=True, stop=True)
        w1s = work.tile([128, 2 * LC], BF, tag="w1s")
        for g in range(2):
            nc.vector.tensor_scalar(w1s[:, LC * g:LC * (g + 1)],
                                    gsegB[:, LC * g:LC * (g + 1)],
                                    col("svecn" + s, j=g),
                                    col("svec" + s, j=g),
                                    op0=OP.mult, op1=OP.add)
        Qcs = work.tile([128, 2 * LC], BF, tag="Qcs")
        nc.vector.tensor_tensor(Qcs[:], Qt[:], w1s[:], op=OP.mult)

        # exp((Qt - gmax)/T2) with pad rows forced to ~0 via padneg bias
        def gmax_exp(srcT, tag, W):
            from concourse import bass_isa
            fm = work.tile([128, 1], FP, tag=tag + "fm")
            nc.vector.tensor_reduce(fm[:], srcT[:], axis=AX.X, op=OP.max)
            gm = work.tile([128, 1], FP, tag=tag + "gm")
            nc.gpsimd.partition_all_reduce(gm[:], fm[:], channels=128,
                                           reduce_op=bass_isa.ReduceOp.max)
            nb = work.tile([128, 1], FP, tag=tag + "nb")
            nc.vector.tensor_scalar(nb[:], gm[:], -1.0 / T2, col("padneg"),
                                    op0=OP.mult, op1=OP.add)
            ex = work.tile([128, W], BF, tag=tag + "ex")
            nc.scalar.activation(ex[:], srcT[:], ACTF.Exp, bias=nb[:],
                                 scale=1.0 / T2)
            return ex

        Qe2 = gmax_exp(Qt, "q2", 2 * LC)

        # ---- assemble full-length hn, then K/V side (all 197 keys) ----
        hnF = work.tile([128, L], BF, tag="hnF")
        if _FAKE_GATHER == 2:
            nc.scalar.activation(hnF[:, 0:98], hn[:, 0:98], ACTF.Copy)
            nc.scalar.activation(hnF[:, 98:L], hn[:, 0:LC], ACTF.Copy)
        else:
            nc.sync.dma_start(hnF[:, 0:98], gout[0:128, 0:98])
            nc.sync.dma_start(hnF[:, 98:L], gout[128:256, :])
        mxF = work.tile([128, L], BF, tag="mxBf", name=f"mxK{i}")
        xeK = trop_exp_rhs(hnF, mxF, L)
        Kt = work.tile([128, 2 * L], BF, tag="Kt")
        for g in range(2):
            trop_project(xeK, mxF, wpb("wkexpT" + s)[:, 128 * g:128 * (g + 1)],
                         col("ck" + s, j=g), Kt[:, L * g:L * (g + 1)], L)
        Ke2 = gmax_exp(Kt, "k2", 2 * L)

        # V in padded layout: Vsb[0:kn, 256t+128g+32j : +16] = V head
        Vsb = work.tile([128, 512], BF, tag="Vsb")
        for t, (k0, kn) in enumerate(((0, KT0), (KT0, KT1))):
            for g in range(2):
                vp = pp([128, 128])
                nc.tensor.matmul(vp[0:kn, :], hnF[:, k0:k0 + kn],
                                 wpb("wvT" + s)[:, 128 * g:128 * (g + 1)],
                                 start=True, stop=True)
                nc.scalar.activation(
                    Vsb[0:kn, 256 * t + 128 * g:256 * t + 128 * (g + 1)],
                    vp[0:kn, :], ACTF.Copy)

        sig8 = psp.tile([8, LC], FP, tag="sig8", padded_shape=[128, 512])
        eo, _ = PKB.sl("e8")
        oSums = []
        for g in range(2):
            oA = psp.tile([128, LC], FP, tag="oA", padded_shape=[128, 512], name=f"oA{i}{g}")
            oB = psp.tile([128, LC], FP, tag="oB", padded_shape=[128, 512], name=f"oB{i}{g}")
            for j in range(4):
                h = 4 * g + j
                ps32 = slice(32 * j, 32 * (j + 1))
                gB = pp([128, LC])
                nc.tensor.matmul(gB[:],
                                 wpb("oneh", rows=8)[:, 128 * h:128 * (h + 1)],
                                 gts[:], start=True, stop=True)
                for t, (k0, kn) in enumerate(((0, KT0), (KT0, KT1))):
                    sts = pp([128, LC])
                    nc.tensor.matmul(sts[0:kn, :],
                                     Ke2[ps32, L * g + k0:L * g + k0 + kn],
                                     Qe2[ps32, LC * g:LC * (g + 1)],
                                     start=True, stop=True,
                                     tile_position=(32 * j, 0))
                    scs = pp([128, LC])
                    nc.tensor.matmul(scs[0:kn, :],
                                     Kt[ps32, L * g + k0:L * g + k0 + kn],
                                     Qcs[ps32, LC * g:LC * (g + 1)],
                                     start=True, stop=True,
                                     tile_position=(32 * j, 0))
                    lg = work.tile([128, LC], FP, tag="sc_lg")
                    nc.scalar.activation(lg[0:kn, :], sts[0:kn, :], ACTF.Ln,
                                         bias=col("clampcol", rows=kn))
                    u = work.tile([128, LC], FP, tag="sc_u")
                    nc.vector.tensor_tensor(u[0:kn, :], lg[0:kn, :],
                                            gB[0:kn, :], op=OP.mult)
                    u2 = work.tile([128, LC], FP, tag="sc_u2")
                    nc.vector.tensor_tensor(u2[0:kn, :], u[0:kn, :],
                                            scs[0:kn, :], op=OP.add)
                    Pt = work.tile([128, LC], BF, tag="sc_P")
                    nc.scalar.activation(Pt[0:kn, :], u2[0:kn, :], ACTF.Exp,
                                         bias=col("zerocol", rows=kn))
                    first = (h == 0 and t == 0)
                    last = (h == H - 1 and t == 1)
                    nc.tensor.matmul(
                        sig8[:], WB[0:kn, eo + 8 * h:eo + 8 * h + 8],
                        Pt[0:kn, :], start=first, stop=last)
                    ot = oA if t == 0 else oB
                    nc.tensor.matmul(
                        ot[ps32, :],
                        Vsb[0:kn, 256 * t + 128 * g + 32 * j:
                            256 * t + 128 * g + 32 * (j + 1)],
                        Pt[0:kn, :], start=True, stop=True,
                        tile_position=(0, 32 * j))
            oSum = work.tile([128, LC], FP, tag="oSum", name=f"oSum{i}{g}")
            nc.scalar.activation(oSum[:], oA[:], ACTF.Copy)
            nc.vector.tensor_tensor(oSum[:], oSum[:], oB[:], op=OP.add)
            oSums.append(oSum)
        rs8 = work.tile([8, LC], FP, tag="rs8")
        nc.vector.reciprocal_approx_fast(out=rs8[:], in_=sig8[:])
        rs8b = work.tile([8, LC], BF, tag="rs8b")
        nc.scalar.activation(rs8b[:], rs8[:], ACTF.Copy)
        pj = pp([128, LC])
        for g in range(2):
            rsB = pp([128, LC])
            nc.tensor.matmul(rsB[:], wpb("segg", rows=8)[:, 128 * g:128 * (g + 1)],
                             rs8b[:], start=True, stop=True)
            onrm = work.tile([128, LC], BF, tag="onrm")
            nc.vector.tensor_tensor(onrm[:], oSums[g][:], rsB[:], op=OP.mult)
            nc.tensor.matmul(pj[:], wpb("woT" + s)[:, 128 * g:128 * (g + 1)],
                             onrm[:], start=(g == 0), stop=(g == 1))
        nc.vector.scalar_tensor_tensor(hT[:, 0:LC], pj[:], col("bo2" + s),
                                       hT[:, 0:LC], op0=OP.add, op1=OP.add)

        # ---- FFN ----
        hn2 = layer_norm(wp("ln2wb" + s), BF)
        mxB2 = work.tile([128, LC], BF, tag="mxBq", name=f"mxF{i}")
        xe2 = trop_exp_rhs(hn2, mxB2, LC)
        dp = psp.tile([128, LC], FP, tag="dp", padded_shape=[128, 512])
        trops, clss, gfs = [], [], []
        for t in range(2):
            # z = T1*ln(sum exp) + ctu + mx  (fp32 for the piecewise fold)
            spz = pp([128, LC])
            nc.tensor.matmul(spz[:], wpb("tuexpT" + s)[:, 128 * t:128 * (t + 1)],
                             xe2[:], start=True, stop=True)
            lgf = work.tile([128, LC], FP, tag="lgf")
            nc.scalar.activation(lgf[:], spz[:], ACTF.Ln, bias=col("clampcol"))
            mxc = work.tile([128, LC], FP, tag="mxc")
            nc.vector.tensor_scalar(mxc[:], mxB2[:], col("ctu" + s, j=t), None,
                                    op0=OP.add)
            zT = work.tile([128, LC], FP, tag="zT")
            nc.vector.scalar_tensor_tensor(zT[:], lgf[:], col("t1col"), mxc[:],
                                           op0=OP.mult, op1=OP.add)
            zmx = work.tile([128, LC], FP, tag="zmx")
            zmn = work.tile([128, LC], FP, tag="zmn")
            nc.vector.tensor_scalar(zmx[:], zT[:], col("lfa" + s, j=t * 8),
                                    col("lfc" + s, j=t * 8), op0=OP.mult,
                                    op1=OP.add)
            nc.vector.tensor_copy(zmn[:], zmx[:])
            for p in range(1, P):
                nc.vector._custom_dve(AFF_MAX, out=zmx[:], in0=zT[:],
                                      in1=zmx[:],
                                      s0=col("lfa" + s, j=t * 8 + p),
                                      s1=col("lfc" + s, j=t * 8 + p))
                nc.vector._custom_dve(AFF_MIN, out=zmn[:], in0=zT[:],
                                      in1=zmn[:],
                                      s0=col("lfa" + s, j=t * 8 + p),
                                      s1=col("lfc" + s, j=t * 8 + p))
            trop_t = work.tile([128, LC], FP, tag="trop_t", name=f"trop{i}{t}")
            nc.vector.tensor_scalar(trop_t[:], zmx[:], col("glc" + s, j=t),
                                    None, op0=OP.mult)
            nc.vector.scalar_tensor_tensor(trop_t[:], zmn[:],
                                           col("glc1m" + s, j=t), trop_t[:],
                                           op0=OP.mult, op1=OP.add)
            trops.append(trop_t)
        for t in range(2):
            # gelu(x) ~= x * sigmoid(1.702 x), sigmoid via Exp + DVE reciprocal
            cp = pp([128, LC])
            nc.tensor.matmul(cp[:], wpb("cuT" + s)[:, 128 * t:128 * (t + 1)],
                             hn2[:], start=True, stop=True)
            eg = work.tile([128, LC], BF, tag="eg")
            nc.scalar.activation(eg[:], cp[:], ACTF.Exp, scale=-1.702,
                                 bias=col("cubn" + s, j=t))
            es1 = work.tile([128, LC], FP, tag="es1")
            nc.vector.tensor_scalar(es1[:], eg[:], 1.0, None, op0=OP.add)
            er = work.tile([128, LC], FP, tag="er")
            nc.vector.reciprocal_approx_fast(out=er[:], in_=es1[:])
            xb = work.tile([128, LC], FP, tag="xb")
            nc.vector.tensor_scalar(xb[:], cp[:], col("cub" + s, j=t), None,
                                    op0=OP.add)
            cls_t = work.tile([128, LC], FP, tag="cls_t", name=f"cls{i}{t}")
            nc.vector.tensor_tensor(cls_t[:], xb[:], er[:], op=OP.mult)
            clss.append(cls_t)
        for t in range(2):
            fgp = pp([128, LC])
            nc.tensor.matmul(fgp[:], wpb("fgT" + s)[:, 128 * t:128 * (t + 1)],
                             hn2[:], start=True, stop=True)
            fe = work.tile([128, LC], BF, tag="fe")
            nc.scalar.activation(fe[:], fgp[:], ACTF.Exp, scale=-1.0,
                                 bias=col("fgbn" + s, j=t))
            fs1 = work.tile([128, LC], FP, tag="fs1")
            nc.vector.tensor_scalar(fs1[:], fe[:], 1.0, None, op0=OP.add)
            gf = work.tile([128, LC], FP, tag="gf", name=f"gf{i}{t}")
            nc.vector.reciprocal_approx_fast(out=gf[:], in_=fs1[:])
            gfs.append(gf)
        for t in range(2):
            dt_ = work.tile([128, LC], FP, tag="dt_")
            nc.vector.tensor_tensor(dt_[:], trops[t][:], clss[t][:],
                                    op=OP.subtract)
            fmid = work.tile([128, LC], FP, tag="fmid")
            nc.vector.tensor_tensor(fmid[:], gfs[t][:], dt_[:], op=OP.mult)
            fused = work.tile([128, LC], BF, tag="fused")
            nc.vector.tensor_tensor(fused[:], fmid[:], clss[t][:], op=OP.add)
            nc.tensor.matmul(dp[:], wpb("dnT" + s)[:, 128 * t:128 * (t + 1)],
                             fused[:], start=(t == 0), stop=(t == 1))
        nc.vector.scalar_tensor_tensor(hT[:, 0:LC], dp[:], col("dnb" + s),
                                       hT[:, 0:LC], op0=OP.add, op1=OP.add)

    # ---- final LN (cls column only) + head ----
    h0 = work.tile([128, 1], FP, tag="h0")
    nc.vector.tensor_copy(h0[:], hT[:, 0:1])
    sq0 = work.tile([128, 1], FP, tag="sq0")
    nc.vector.tensor_tensor(sq0[:], h0[:], h0[:], op=OP.mult)
    st0 = pp([1, 2])
    nc.tensor.matmul(st0[0:1, 0:1], meancol, h0[:], start=True, stop=True)
    nc.tensor.matmul(st0[0:1, 1:2], meancol, sq0[:], start=True, stop=True)
    mean0 = work.tile([1, 2], FP, tag="mean0")
    nc.vector.tensor_copy(mean0[:], st0[0:1, 0:2])
    var0 = work.tile([1, 1], FP, tag="var0")
    nc.vector.tensor_tensor(var0[:], mean0[0:1, 0:1], mean0[0:1, 0:1],
                            op=OP.mult)
    nc.vector.tensor_tensor(var0[:], mean0[0:1, 1:2], var0[:], op=OP.subtract)
    lnv0 = work.tile([1, 1], FP, tag="lnv0")
    nc.scalar.activation(lnv0[:], var0[:], ACTF.Ln, bias=col("epscol", rows=1))
    rstd0 = work.tile([1, 1], FP, tag="rstd0")
    nc.scalar.activation(rstd0[:], lnv0[:], ACTF.Exp, scale=-0.5)
    mrow = work.tile([1, 2], FP, tag="mrow")
    nc.vector.tensor_tensor(mrow[0:1, 0:1], mean0[0:1, 0:1], rstd0[:],
                            op=OP.mult)
    nc.vector.tensor_copy(mrow[0:1, 1:2], rstd0[:])
    mB = pp([128, 2])
    nc.tensor.matmul(mB[:], onesrow, mrow[:], start=True, stop=True)
    t0 = work.tile([128, 1], FP, tag="t0")
    nc.vector.tensor_tensor(t0[:], h0[:], mB[:, 1:2], op=OP.mult)
    nc.vector.tensor_tensor(t0[:], t0[:], mB[:, 0:1], op=OP.subtract)
    hf = work.tile([128, 1], FP, tag="hf")
    nc.vector.tensor_scalar(hf[:], t0[:], wp("fnwb")[:, 0:1],
                            wp("fnwb")[:, 1:2], op0=OP.mult, op1=OP.add)
    hd = pp([128, 8])
    for j in range(8):
        nc.tensor.matmul(hd[:, j:j + 1], wp("hwT")[:, 128 * j:128 * (j + 1)],
                         hf[:], start=True, stop=True)
    ob = work.tile([128, 8], FP, tag="ob")
    nc.vector.tensor_tensor(ob[:], hd[:], wp("hb"), op=OP.add)
    if reps > 1:
        loop_cm.__exit__(None, None, None)
    nc.sync.dma_start(out_d[:], ob[:])
    ctx.close()


# ---------------------------------------------------------------- entry point
_NC_CACHE = []


def _get_nc():
    if not _NC_CACHE:
        _NC_CACHE.append(_build_program())
    return _NC_CACHE[0]


def build_in_maps(inputs):
    inputs = {k: np.asarray(v, np.float32) for k, v in inputs.items()}
    Wsh, WBsh, b0 = _prep_pack(inputs)
    B = inputs["x"].shape[0]
    bfnp = mybir.dt.np(BF)
    in_maps = []
    o, n = PKB.sl("xp")
    ob0, _ = PK.sl("b0")
    for c in range(8):
        img, rank = c // 2, c % 2
        Wc = Wsh.copy()
        Wc[:, ob0:ob0 + LC] = b0[:, 0:LC] if rank == 0 else b0[:, 98:98 + LC]
        WBc = WBsh.copy()
        WBc[:, o:o + n] = _prep_x(inputs, img % B, rank)
        in_maps.append({"inp": Wc, "inpb": WBc.astype(bfnp)})
    return in_maps


def _gather_outs(res_list, B):
    outs = []
    for b in range(B):
        om = res_list[2 * b]["out"]
        outs.append(np.asarray(om).T.reshape(-1)[:NC])
    return np.stack(outs).astype(np.float32)


def kernel(**inputs):
    nc = _get_nc()
    in_maps = build_in_maps(inputs)
    B = np.asarray(inputs["x"]).shape[0]
    res = bass_utils.run_bass_kernel_spmd(nc, in_maps, core_ids=list(range(8)))
    return _gather_outs(res.results, B)

